# revision 15
# baseline (speedup 1.0000x reference)
"""Trainium2 Bass kernel for nn_C2f_DualModal_MoE (C2f block with top-1 MoE routing).

Strategy (data-parallel over batch, 4 samples per core on 8 cores):
  - all matmuls in bf16 (same PE rate as f32r but ~60 fewer overhead cycles
    per matmul: FWL fast-weight-load triggers for non-fp32 dtypes), with
    redundant LDWEIGHTS for consecutive same-weight matmuls dropped
    post-compile (each conv tap / cv2 chunk loads once per ii-pair);
  - cv1 (1x1 conv 256->256 + SiLU) as bf16 matmuls over 400-pixel tiles;
    the `feat` half is written into a zero-padded [82x82] spatial layout so
    the 3x3 convs become 9 shift-offset matmuls. The global-average-pool for
    the router comes free via the activation accum_out.
  - Router: tiny f32 matmul + softmax on-chip; the top-1 selection is turned
    into a one-hot vector (no control flow), which selects the routed expert's
    weights via 3 vector ops (Wsel = sum_e onehot[e] * We[e]); since top-1,
    conv(feat, Wsel) == conv(feat, We[argmax]).
  - shared + routed 3x3 convs (SiLU), moe = shared + gate * routed.
  - cv2 (1x1 conv 384->256 + SiLU) fused per tile from (a, feat, moe) without
    materializing the concat (chunk-outer order for weight reuse); routed-conv
    and cv2 are software-pipelined by one tile.
  - router softmax uses tanh ((1+t)/(1-t) identity) instead of exp so the ACT
    engine never swaps its activation table away from the Silu set.
Matmuls bf16, accumulation f32; rel err ~3.6e-3 vs the 2e-2 gate.
"""

import numpy as np

import concourse.bass as bass
import concourse.bacc as bacc
import concourse.tile as tile
from concourse import mybir
from concourse.bass_utils import run_bass_kernel_spmd

# Problem constants (hardcoded per contract)
B, C1, C2 = 32, 256, 256
H = W = 80
CH = 128
NE = 3
NCORES = 8
BPC = B // NCORES          # samples per core = 4
NPIX = H * W               # 6400
PADW = W + 2               # 82
PADH = H + 2               # 82
RPT = 5                    # rows per pixel tile
TN = RPT * W               # 400 pixels per tile
NT = H // RPT              # 16 tiles
NP = NT // 2               # 8 tile-pairs
TAPS = [(dy, dx) for dy in range(3) for dx in range(3)]

f32 = mybir.dt.float32
f32r = mybir.dt.float32r
bf16 = mybir.dt.bfloat16


def _emit(nc, tc, ctx, reps=1, sim_compat=False, tune=None, internal_io=False):
    AX = mybir.AxisListType
    OP = mybir.AluOpType
    AF = mybir.ActivationFunctionType
    tune = {**dict(xbufs=4, obufs=4, rbufs=2, psbufs=3, fpdouble=True,
                   adouble=True, bf16=True), **(tune or {})}
    dmm = bf16 if tune["bf16"] else f32r

    io_kind = "Internal" if internal_io else "ExternalInput"
    x_d = nc.dram_tensor("x", [BPC, 2, CH, NPIX], dmm, kind=io_kind).ap()
    w1_d = nc.dram_tensor("w1t", [2, CH, 2 * CH], dmm, kind="ExternalInput").ap()
    b1_d = nc.dram_tensor("b1r", [2, CH], f32, kind="ExternalInput").ap()
    wr_d = nc.dram_tensor("wrs", [CH, NE], f32, kind="ExternalInput").ap()
    br_d = nc.dram_tensor("brr", [1, NE], f32, kind="ExternalInput").ap()
    ws_d = nc.dram_tensor("wst", [CH, 9 * CH], dmm, kind="ExternalInput").ap()
    bs_d = nc.dram_tensor("bsr", [CH, 1], f32, kind="ExternalInput").ap()
    we_d = nc.dram_tensor("wet", [NE, CH, 9 * CH], f32, kind="ExternalInput").ap()
    be_d = nc.dram_tensor("ber", [CH, NE], f32, kind="ExternalInput").ap()
    w2_d = nc.dram_tensor("w2t", [3, CH, C2], dmm, kind="ExternalInput").ap()
    b2_d = nc.dram_tensor("b2r", [2, CH], f32, kind="ExternalInput").ap()
    y_d = nc.dram_tensor(
        "y", [BPC, 2, CH, NPIX], f32,
        kind="Internal" if internal_io else "ExternalOutput").ap()

    wpool = ctx.enter_context(tc.tile_pool(name="weights", bufs=1))
    ppool = ctx.enter_context(tc.tile_pool(name="persist", bufs=1))
    xpool = ctx.enter_context(tc.tile_pool(name="xin", bufs=tune["xbufs"]))
    opool = ctx.enter_context(tc.tile_pool(name="oout", bufs=tune["obufs"]))
    rpool = ctx.enter_context(tc.tile_pool(name="rtile", bufs=tune["rbufs"]))
    spool = ctx.enter_context(tc.tile_pool(name="small", bufs=2))
    selpool = ctx.enter_context(tc.tile_pool(name="sel", bufs=1))
    psum = ctx.enter_context(tc.tile_pool(name="psum", bufs=tune["psbufs"], space="PSUM"))
    psumS = ctx.enter_context(tc.tile_pool(name="psumS", bufs=1, space="PSUM"))

    # ---- load weights into SBUF (resident) ----
    w1_sb = wpool.tile([CH, 2 * 2 * CH], dmm)
    for k in range(2):
        nc.sync.dma_start(w1_sb[:, k * 256:(k + 1) * 256], w1_d[k])
    ws_sb = wpool.tile([CH, 9 * CH], dmm)
    nc.sync.dma_start(ws_sb[:], ws_d)
    we_sb = wpool.tile([CH, NE * 9 * CH], f32)
    for e in range(NE):
        nc.sync.dma_start(we_sb[:, e * 1152:(e + 1) * 1152], we_d[e])
    w2_sb = wpool.tile([CH, 3 * C2], dmm)
    for k in range(3):
        nc.sync.dma_start(w2_sb[:, k * 256:(k + 1) * 256], w2_d[k])
    wr_sb = wpool.tile([CH, NE], f32)
    nc.sync.dma_start(wr_sb[:], wr_d)
    br_sb = wpool.tile([1, NE], f32)
    nc.sync.dma_start(br_sb[:], br_d)
    bs_sb = wpool.tile([CH, 1], f32)
    nc.sync.dma_start(bs_sb[:], bs_d)
    be_sb = wpool.tile([CH, NE], f32)
    nc.sync.dma_start(be_sb[:], be_d)
    b1_sb = wpool.tile([CH, 2], f32)
    for k in range(2):
        nc.sync.dma_start(b1_sb[:, k:k + 1], b1_d[k])
    b2_sb = wpool.tile([CH, 2], f32)
    for k in range(2):
        nc.sync.dma_start(b2_sb[:, k:k + 1], b2_d[k])
    ones_sb = wpool.tile([1, CH], f32)
    nc.vector.memset(ones_sb[:], 1.0)

    if internal_io:
        # timing mode: x is Internal (uninitialized) DRAM; zero it once so
        # the timed loop computes on deterministic, non-denormal data.
        zs = wpool.tile([CH, 800], dmm, name="zs")
        if tune["bf16"]:
            nc.vector.memset(zs[:], 0.0)
        else:
            nc.vector.memset(zs[:].bitcast(f32), 0.0)
        for zb in range(BPC):
            for zk in range(2):
                for zj in range(NPIX // 800):
                    nc.sync.dma_start(
                        x_d[zb, zk, :, zj * 800:(zj + 1) * 800], zs[:])

    # ---- persistent per-sample working buffers ----
    # (optionally double-buffered across samples to decouple next-sample cv1
    # writes from current-sample conv/cv2 reads)
    fps = []
    for fi in range(2 if tune["fpdouble"] else 1):
        fp = ppool.tile([CH, PADH * PADW], dmm, tag=f"fp{fi}", name=f"fp{fi}")
        # zero once: borders stay zero forever (bitcast: memset lacks f32r)
        if tune["bf16"]:
            nc.vector.memset(fp[:], 0.0)
        else:
            nc.vector.memset(fp[:].bitcast(f32), 0.0)
        fps.append(fp[:].rearrange("p (r c) -> p r c", c=PADW))
    a_sbs = [ppool.tile([CH, NPIX], dmm, tag=f"a{ai}", name=f"a{ai}")
             for ai in range(2 if tune["adouble"] else 1)]
    sh_sb = ppool.tile([CH, NPIX], bf16)
    moe_sb = ppool.tile([CH, NPIX], dmm)
    # fixed-address x tiles for (b=0, pi=0..1): filled in a prologue before
    # the rep loop and re-filled at the END of each body iteration, so the
    # next rep's cv1 never waits behind this rep's y-store DMA queue.
    xt_pre = []
    for pp in range(4):
        t0 = ppool.tile([CH, 2 * TN], dmm, tag=f"xtp0{pp}", name=f"xtp0{pp}")
        t1 = ppool.tile([CH, 2 * TN], dmm, tag=f"xtp1{pp}", name=f"xtp1{pp}")
        nc.sync.dma_start(t0[:], x_d[0, 0, :, pp * 800:(pp + 1) * 800])
        nc.sync.dma_start(t1[:], x_d[0, 1, :, pp * 800:(pp + 1) * 800])
        xt_pre.append((t0, t1))

    tmpool = ctx.enter_context(tc.tile_pool(name="silutmp", bufs=2)) if sim_compat else None

    def act_silu(out_ap, ps_ap, bias_ap, accum_ap=None):
        """SiLU from PSUM -> SBUF. On HW, one ACT instruction (with optional
        free GAP accumulation). CoreSim lacks Silu, so sim_compat emulates via
        Sigmoid + (ps+bias)*sig, and computes the accumulation separately."""
        if not sim_compat:
            if accum_ap is not None:
                nc.scalar.activation(out_ap, ps_ap, AF.Silu, bias=bias_ap,
                                     scale=1.0, accum_out=accum_ap)
            else:
                nc.scalar.activation(out_ap, ps_ap, AF.Silu, bias=bias_ap,
                                     scale=1.0)
            return
        shp = list(out_ap.shape[1:])
        fs = 1
        for d in shp:
            fs *= d
        tmp = tmpool.tile([CH, 2 * TN], f32, tag="sigmoid_tmp")
        tv = tmp[:, 0:fs]
        if len(shp) == 2:
            tv = tv.rearrange("p (g c) -> p g c", g=shp[0])
        elif len(shp) == 3:
            tv = tv.rearrange("p (g r c) -> p g r c", g=shp[0], r=shp[1])
        nc.scalar.activation(tv, ps_ap, AF.Sigmoid, bias=bias_ap, scale=1.0)
        nc.vector.scalar_tensor_tensor(out_ap, ps_ap, bias_ap, tv,
                                       op0=OP.add, op1=OP.mult)
        if accum_ap is not None:
            axis = [None, AX.X, AX.XY, AX.XYZ][len(shp)]
            nc.vector.reduce_sum(accum_ap, out_ap, axis=axis)

    def conv_tile_matmuls(ps, wsb, i, fp3):
        for t, (dy, dx) in enumerate(TAPS):
            rhs = fp3[:, i * RPT + dy: i * RPT + dy + RPT, dx: dx + W]
            nc.tensor.matmul(
                ps[:],
                wsb[:, t * CH:(t + 1) * CH],
                rhs,
                start=(t == 0),
                stop=(t == 8),
            )

    def _body():
        xt_cache = {}

        def emit_xt(b, pi):
            xt0 = xpool.tile([CH, 2 * TN], dmm, tag="xt0")
            nc.sync.dma_start(xt0[:], x_d[b, 0, :, pi * 800:(pi + 1) * 800])
            xt1 = xpool.tile([CH, 2 * TN], dmm, tag="xt1")
            nc.sync.dma_start(xt1[:], x_d[b, 1, :, pi * 800:(pi + 1) * 800])
            return xt0, xt1

        for b in range(BPC):
            fp3 = fps[b % len(fps)]
            fp3v = fp3  # [128, 82, 82] padded view
            a_sb = a_sbs[b % len(a_sbs)]
            # ---- cv1 over tile-PAIRS: 800 px per ACT, shared-weight MM runs,
            # GAP accumulated for free ----
            gap_sb = spool.tile([CH, NP], f32, tag="gap")
            for pi in range(NP):
                i0 = 2 * pi
                if b == 0 and pi < 4:
                    xt0, xt1 = xt_pre[pi]
                elif (b, pi) in xt_cache:
                    xt0, xt1 = xt_cache.pop((b, pi))
                else:
                    xt0, xt1 = emit_xt(b, pi)
                ps_a = psum.tile([CH, 2, 512], f32, tag="ps")
                ps_f = psum.tile([CH, 2, 512], f32, tag="ps")
                for k, xt in ((0, xt0), (1, xt1)):
                    for hw_, ps2 in ((0, ps_a), (1, ps_f)):
                        wsl = w1_sb[:, k * 256 + hw_ * 128: k * 256 + hw_ * 128 + 128]
                        for ii in range(2):
                            nc.tensor.matmul(ps2[:, ii, 0:TN], wsl,
                                             xt[:, ii * TN:(ii + 1) * TN],
                                             start=(k == 0), stop=(k == 1))
                act_silu(a_sb[:, i0 * TN:(i0 + 2) * TN].rearrange(
                             "p (g c) -> p g c", g=2),
                         ps_a[:, :, 0:TN], b1_sb[:, 0:1])
                fout = fp3v[:, 1 + 10 * pi: 11 + 10 * pi, 1:1 + W].rearrange(
                    "p (g r) c -> p g r c", g=2)
                act_silu(fout,
                         ps_f[:, :, 0:TN].rearrange("p g (r c) -> p g r c", c=W),
                         b1_sb[:, 1:2], accum_ap=gap_sb[:, pi:pi + 1])

            # ---- router: logits -> softmax -> top-1 one-hot + gate ----
            pooled = spool.tile([CH, 1], f32, tag="pooled")
            nc.vector.reduce_sum(pooled[:], gap_sb[:], axis=AX.X)
            ps_l = psumS.tile([1, NE], f32, tag="psl")
            # wr is pre-scaled by 1/NPIX on the host, so sums (not means) work.
            nc.tensor.matmul(ps_l[:], pooled[:], wr_sb[:], start=True, stop=True)
            logits = spool.tile([1, NE], f32, tag="logits")
            nc.vector.tensor_add(logits[:], ps_l[:], br_sb[:])
            m_sb = spool.tile([1, 1], f32, tag="m")
            nc.vector.reduce_max(m_sb[:], logits[:], axis=AX.X)
            negm = spool.tile([1, 1], f32, tag="negm")
            nc.vector.tensor_scalar_mul(negm[:], m_sb[:], -0.5)
            t_sb = spool.tile([1, NE], f32, tag="tsb")
            nc.scalar.activation(t_sb[:], logits[:], AF.Tanh, bias=negm[:],
                                 scale=0.5)
            num = spool.tile([1, NE], f32, tag="num")
            nc.vector.tensor_scalar_add(num[:], t_sb[:], 1.0)
            den = spool.tile([1, NE], f32, tag="den")
            nc.vector.tensor_scalar(den[:], t_sb[:], -1.0, 1.0,
                                    op0=OP.mult, op1=OP.add)
            rden = spool.tile([1, NE], f32, tag="rden")
            nc.vector.reciprocal(rden[:], den[:])
            e_sb = spool.tile([1, NE], f32, tag="esb")
            nc.vector.tensor_mul(e_sb[:], num[:], rden[:])
            s_sb = spool.tile([1, 1], f32, tag="ssb")
            nc.vector.reduce_sum(s_sb[:], e_sb[:], axis=AX.X)
            wgt = spool.tile([1, 1], f32, tag="wgt")
            nc.vector.reciprocal(wgt[:], s_sb[:])
            oh = spool.tile([1, NE], f32, tag="oh")
            nc.vector.tensor_scalar(oh[:], logits[:], m_sb[:], None, op0=OP.is_ge)
            bc = spool.tile([1, NE + 1], f32, tag="bc")
            nc.vector.tensor_copy(bc[:, 0:NE], oh[:])
            nc.vector.tensor_copy(bc[:, NE:NE + 1], wgt[:])
            ps_bc = psumS.tile([CH, NE + 1], f32, tag="psb")
            nc.tensor.matmul(ps_bc[:], ones_sb[:], bc[:], start=True, stop=True)
            sc = spool.tile([CH, NE + 1], f32, tag="sc")
            nc.vector.tensor_copy(sc[:], ps_bc[:])

            # ---- expert-weight select: Wsel = sum_e onehot[e] * We[e] ----
            wA = selpool.tile([CH, 9 * CH], f32, tag="wA")
            nc.vector.tensor_scalar_mul(wA[:], we_sb[:, 0:1152], sc[:, 0:1])
            wB = selpool.tile([CH, 9 * CH], f32, tag="wB")
            nc.vector.scalar_tensor_tensor(wB[:], we_sb[:, 1152:2304], sc[:, 1:2],
                                           wA[:], op0=OP.mult, op1=OP.add)
            wS = selpool.tile([CH, 9 * CH], dmm, tag="wS")
            nc.vector.scalar_tensor_tensor(wS[:], we_sb[:, 2304:3456], sc[:, 2:3],
                                           wB[:], op0=OP.mult, op1=OP.add)
            bA = spool.tile([CH, 1], f32, tag="bA")
            nc.vector.tensor_scalar_mul(bA[:], be_sb[:, 0:1], sc[:, 0:1])
            bB = spool.tile([CH, 1], f32, tag="bB")
            nc.vector.scalar_tensor_tensor(bB[:], be_sb[:, 1:2], sc[:, 1:2],
                                           bA[:], op0=OP.mult, op1=OP.add)
            bS = spool.tile([CH, 1], f32, tag="bS")
            nc.vector.scalar_tensor_tensor(bS[:], be_sb[:, 2:3], sc[:, 2:3],
                                           bB[:], op0=OP.mult, op1=OP.add)

            # prefetch next sample's first x pairs ahead of this sample's
            # y stores in the DMA queue (slots are free: cv1(b) is done)
            if b + 1 < BPC:
                for pp in range(2):
                    xt_cache[(b + 1, pp)] = emit_xt(b + 1, pp)
            else:
                # refill the loop-carried preload tiles for the next rep's
                # first sample (reads of this rep's b=0 are long done)
                for pp in range(4):
                    t0, t1 = xt_pre[pp]
                    nc.sync.dma_start(t0[:], x_d[0, 0, :, pp * 800:(pp + 1) * 800])
                    nc.sync.dma_start(t1[:], x_d[0, 1, :, pp * 800:(pp + 1) * 800])

            def conv_pair(ps2, wsb, pi):
                i0 = 2 * pi
                for t, (dy, dx) in enumerate(TAPS):
                    wt = wsb[:, t * CH:(t + 1) * CH]
                    for ii in range(2):
                        rhs = fp3[:, (i0 + ii) * RPT + dy: (i0 + ii) * RPT + dy + RPT,
                                  dx: dx + W]
                        nc.tensor.matmul(ps2[:, ii, 0:TN], wt, rhs,
                                         start=(t == 0), stop=(t == 8))

            # ---- shared expert 3x3 conv + SiLU (pairs) ----
            for pi in range(NP):
                ps2 = psum.tile([CH, 2, 512], f32, tag="ps")
                conv_pair(ps2, ws_sb, pi)
                act_silu(sh_sb[:, pi * 800:(pi + 1) * 800].rearrange(
                             "p (g c) -> p g c", g=2),
                         ps2[:, :, 0:TN], bs_sb[:])

            # ---- routed conv + moe + fused cv2, software-pipelined by 1 pair ----
            def cv2_pair(pi):
                i0 = 2 * pi
                for h in range(2):
                    po = psum.tile([CH, 2, 512], f32, tag="ps")
                    for c, src_of in ((0, None), (1, None), (2, None)):
                        wsl = w2_sb[:, c * 256 + h * 128: c * 256 + h * 128 + 128]
                        for ii in range(2):
                            i = i0 + ii
                            if c == 0:
                                rhs = a_sb[:, i * TN:(i + 1) * TN]
                            elif c == 1:
                                rhs = fp3[:, i * RPT + 1: i * RPT + 1 + RPT,
                                          1: 1 + W]
                            else:
                                rhs = moe_sb[:, i * TN:(i + 1) * TN]
                            nc.tensor.matmul(po[:, ii, 0:TN], wsl, rhs,
                                             start=(c == 0), stop=(c == 2))
                    ot = opool.tile([CH, 2 * TN], f32, tag="ot")
                    act_silu(ot[:].rearrange("p (g c) -> p g c", g=2),
                             po[:, :, 0:TN], b2_sb[:, h:h + 1])
                    dq = nc.scalar if (b == BPC - 1 and pi >= NP - 2) else nc.sync
                    dq.dma_start(y_d[b, h, :, pi * 800:(pi + 1) * 800], ot[:])

            for pi in range(NP):
                ps2 = psum.tile([CH, 2, 512], f32, tag="ps")
                conv_pair(ps2, wS, pi)
                rt = rpool.tile([CH, 2 * TN], bf16, tag="rt")
                act_silu(rt[:].rearrange("p (g c) -> p g c", g=2),
                         ps2[:, :, 0:TN], bS[:])
                nc.vector.scalar_tensor_tensor(
                    moe_sb[:, pi * 800:(pi + 1) * 800], rt[:], sc[:, NE:NE + 1],
                    sh_sb[:, pi * 800:(pi + 1) * 800], op0=OP.mult, op1=OP.add)
                if pi > 0:
                    cv2_pair(pi - 1)
            cv2_pair(NP - 1)

    if reps == 1:
        _body()
    else:
        # HW timing mode: repeat the whole workload in a hardware loop
        # (same instruction count / compile cost; R x device work).
        with tc.For_i(0, reps, 1):
            _body()
    if internal_io:
        # tiny external output so the (otherwise internal-IO) program is not
        # dead-code eliminated; depends on the looped work via y.
        ydig_d = nc.dram_tensor("ydig", [CH, 4], f32,
                                kind="ExternalOutput").ap()
        ydig_t = opool.tile([CH, 4], f32, name="ydig_t")
        nc.sync.dma_start(ydig_t[:], y_d[0, 0, :, 0:4])
        nc.sync.dma_start(ydig_d, ydig_t[:])


def _ldw_key(ins):
    w = ins.ins[0]
    return (str(getattr(w, "memref", None)), str(getattr(w, "ap", None)),
            getattr(w, "offset", None), str(getattr(w, "dtype", None)),
            str(ins.perf_mode), str(ins.is_transpose))


def dedupe_ldweights(nc):
    """Drop InstLdweights that reload the identical weights as the previous
    Ldweights in the block (adjacent L-M pattern only); move their
    waits/updates onto the immediately-following matmul. The PE keeps the
    stationary operand across matmuls, so the reload is pure overhead."""
    ndrop = 0
    for blk in nc.main_func.blocks:
        out = []
        last_key = None
        pend = None  # dropped ldweights whose sync must move to next matmul
        for ins in blk.instructions:
            if isinstance(ins, mybir.InstLdweights):
                key = _ldw_key(ins)
                if key == last_key and pend is None:
                    pend = ins
                    ndrop += 1
                    continue
                last_key = key
            elif isinstance(ins, mybir.InstMatmult):
                if pend is not None:
                    si, pi = ins.sync_info, pend.sync_info
                    if pi is not None and si is not None:
                        for w in list(pi.on_wait or []):
                            si.on_wait.append(w)
                        for u in list(pi.on_update or []):
                            si.on_update.append(u)
                    pend = None
                if ins.ldweights is not False:
                    last_key = None  # self-loading matmul changes PE weights
            elif pend is not None:
                # something else between the dropped L and its M: bail out,
                # restore the load to stay safe.
                out.append(pend)
                pend = None
                ndrop -= 1
            out.append(ins)
        assert pend is None
        blk.instructions[:] = out
    return ndrop


def build(reps=1, sim_compat=False, tune=None, internal_io=False):
    from contextlib import ExitStack
    nc = bacc.Bacc("TRN2", target_bir_lowering=False, debug=False,
                   num_devices=NCORES)
    with tile.TileContext(nc) as tc:
        with ExitStack() as ctx:
            _emit(nc, tc, ctx, reps=reps, sim_compat=sim_compat, tune=tune,
                  internal_io=internal_io)
    nc.compile()
    dedupe_ldweights(nc)
    return nc


def round_f32r(a):
    """Round fp32 to the PE's fp32r format: 11 explicit mantissa bits
    (round-to-nearest-even), low 12 bits zero. The result is both a valid
    fp32 value and a valid fp32r bit pattern."""
    a = np.ascontiguousarray(np.asarray(a, np.float32))
    bits = a.view(np.uint32).astype(np.uint64)
    lsb = (bits >> 12) & 1
    r = (bits + 0x7FF + lsb) & 0xFFFFF000
    return r.astype(np.uint32).view(np.float32)


def marshal_inputs(x, w1, b1, wr, br, ws, bs, we, be, w2, b2, use_bf16=True):
    """Host-side (tiny) weight re-layouts into matmul-friendly forms."""
    asf = lambda a: np.ascontiguousarray(np.asarray(a, dtype=np.float32))
    if use_bf16:
        import ml_dtypes
        cvt = lambda a: np.ascontiguousarray(
            np.asarray(a, np.float32).astype(ml_dtypes.bfloat16))
    else:
        cvt = round_f32r
    x = cvt(x)
    w1t = asf(np.asarray(w1, np.float32).reshape(2 * CH, C1).T.reshape(2, CH, 2 * CH))
    b1r = asf(np.asarray(b1, np.float32).reshape(2, CH))
    wrs = asf(np.asarray(wr, np.float32) / NPIX)
    brr = asf(np.asarray(br, np.float32).reshape(1, NE))
    wst = asf(np.asarray(ws, np.float32).transpose(1, 2, 3, 0).reshape(CH, 9 * CH))
    bsr = asf(np.asarray(bs, np.float32).reshape(CH, 1))
    wet = asf(np.asarray(we, np.float32).transpose(0, 2, 3, 4, 1).reshape(NE, CH, 9 * CH))
    ber = asf(np.asarray(be, np.float32).T)
    w2t = asf(np.asarray(w2, np.float32).reshape(C2, 3 * CH).T.reshape(3, CH, C2))
    b2r = asf(np.asarray(b2, np.float32).reshape(2, CH))
    w1t = cvt(w1t)
    wst = cvt(wst)
    wet = round_f32r(wet) if not use_bf16 else wet
    w2t = cvt(w2t)
    shared = dict(w1t=w1t, b1r=b1r, wrs=wrs, brr=brr, wst=wst, bsr=bsr,
                  wet=wet, ber=ber, w2t=w2t, b2r=b2r)
    xc = x.reshape(NCORES, BPC, 2, CH, NPIX)
    in_maps = [dict(shared, x=np.ascontiguousarray(xc[c])) for c in range(NCORES)]
    return in_maps


_CACHE = {}


def _get_nc():
    if "nc" not in _CACHE:
        _CACHE["nc"] = build(reps=1)
    return _CACHE["nc"]


def _get_runner():
    """Build the sharded PJRT callable once (mirrors
    bass2jax.run_bass_via_pjrt's multi-core path) so repeat kernel() calls
    skip the jax retrace/compile."""
    if "runner" in _CACHE:
        return _CACHE["runner"]
    import jax
    from jax.experimental.shard_map import shard_map
    from jax.sharding import Mesh, PartitionSpec
    from concourse import bass2jax

    nc = _get_nc()
    bass2jax.install_neuronx_cc_hook()
    part_name = nc.partition_id_tensor.name if nc.partition_id_tensor else None
    in_names, out_names, out_avals = [], [], []
    for alloc in nc.m.functions[0].allocations:
        if not isinstance(alloc, mybir.MemoryLocationSet):
            continue
        name = alloc.memorylocations[0].name
        if alloc.kind == "ExternalInput":
            if name != part_name:
                in_names.append(name)
        elif alloc.kind == "ExternalOutput":
            out_names.append(name)
            out_avals.append(jax.core.ShapedArray(
                tuple(alloc.tensor_shape), mybir.dt.np(alloc.dtype)))
    assert nc.dbg_addr is None
    n_params = len(in_names)
    all_in = in_names + out_names  # zero buffers donated as outputs
    if part_name is not None:
        all_in = all_in + [part_name]

    def _body(*args):
        operands = list(args)
        if part_name is not None:
            operands.append(bass2jax.partition_id_tensor())
        outs = bass2jax._bass_exec_p.bind(
            *operands, out_avals=tuple(out_avals), in_names=tuple(all_in),
            out_names=tuple(out_names), lowering_input_output_aliases=(),
            sim_require_finite=True, sim_require_nnan=True, nc=nc)
        return tuple(outs)

    devices = jax.devices()[:NCORES]
    mesh = Mesh(np.asarray(devices), ("core",))
    nio = n_params + len(out_names)
    sharded = jax.jit(
        shard_map(_body, mesh=mesh, in_specs=(PartitionSpec("core"),) * nio,
                  out_specs=(PartitionSpec("core"),) * len(out_names),
                  check_rep=False),
        donate_argnums=tuple(range(n_params, nio)), keep_unused=True)
    _CACHE["runner"] = (sharded, in_names, out_names, out_avals)
    return _CACHE["runner"]


def kernel(x, w1, b1, wr, br, ws, bs, we, be, w2, b2):
    in_maps = marshal_inputs(x, w1, b1, wr, br, ws, bs, we, be, w2, b2)
    sharded, in_names, out_names, out_avals = _get_runner()
    concat_in = [
        np.concatenate([in_maps[c][name] for c in range(NCORES)], axis=0)
        for name in in_names
    ]
    concat_zeros = [
        np.zeros((NCORES * a.shape[0], *a.shape[1:]), a.dtype) for a in out_avals
    ]
    out_arrs = sharded(*concat_in, *concat_zeros)
    y = np.asarray(out_arrs[out_names.index("y")])
    return np.ascontiguousarray(y.reshape(B, C2, H, W))



# revision 16
# speedup vs baseline: 1.1836x; 1.1836x over previous
"""Trainium2 Bass kernel for nn_C2f_DualModal_MoE (C2f block with top-1 MoE routing).

Strategy (data-parallel over batch, 4 samples per core on 8 cores):
  - all matmuls in bf16 (same PE rate as f32r but ~60 fewer overhead cycles
    per matmul: FWL fast-weight-load triggers for non-fp32 dtypes), with
    redundant LDWEIGHTS for consecutive same-weight matmuls dropped
    post-compile (each conv tap / cv2 chunk loads once per ii-pair);
  - cv1 (1x1 conv 256->256 + SiLU) as bf16 matmuls over 400-pixel tiles;
    the `feat` half is written into a zero-padded [82x82] spatial layout so
    the 3x3 convs become 9 shift-offset matmuls. The global-average-pool for
    the router comes free via the activation accum_out.
  - Router: tiny f32 matmul + softmax on-chip; the top-1 selection is turned
    into a one-hot vector (no control flow), which selects the routed expert's
    weights via 3 vector ops (Wsel = sum_e onehot[e] * We[e]); since top-1,
    conv(feat, Wsel) == conv(feat, We[argmax]).
  - shared + routed 3x3 convs (SiLU), moe = shared + gate * routed.
  - cv2 (1x1 conv 384->256 + SiLU) fused per tile from (a, feat, moe) without
    materializing the concat (chunk-outer order for weight reuse); routed-conv
    and cv2 are software-pipelined by one tile.
  - router softmax uses tanh ((1+t)/(1-t) identity) instead of exp so the ACT
    engine never swaps its activation table away from the Silu set.
Matmuls bf16, accumulation f32; rel err ~3.6e-3 vs the 2e-2 gate.
"""

import numpy as np

import concourse.bass as bass
import concourse.bacc as bacc
import concourse.tile as tile
from concourse import mybir
from concourse.bass_utils import run_bass_kernel_spmd

# Problem constants (hardcoded per contract)
B, C1, C2 = 32, 256, 256
H = W = 80
CH = 128
NE = 3
NCORES = 8
BPC = B // NCORES          # samples per core = 4
NPIX = H * W               # 6400
PADW = W + 2               # 82
PADH = H + 2               # 82
RPT = 5                    # rows per pixel tile
TN = RPT * W               # 400 pixels per tile
NT = H // RPT              # 16 tiles
NP = NT // 2               # 8 tile-pairs
TAPS = [(dy, dx) for dy in range(3) for dx in range(3)]

f32 = mybir.dt.float32
f32r = mybir.dt.float32r
bf16 = mybir.dt.bfloat16


def _emit(nc, tc, ctx, reps=1, sim_compat=False, tune=None, internal_io=False):
    AX = mybir.AxisListType
    OP = mybir.AluOpType
    AF = mybir.ActivationFunctionType
    tune = {**dict(xbufs=4, obufs=4, rbufs=2, psbufs=3, fpdouble=True,
                   adouble=True, bf16=True), **(tune or {})}
    dmm = bf16 if tune["bf16"] else f32r

    io_kind = "Internal" if internal_io else "ExternalInput"
    x_d = nc.dram_tensor("x", [BPC, 2, CH, NPIX], dmm, kind=io_kind).ap()
    w1_d = nc.dram_tensor("w1t", [2, CH, 2 * CH], dmm, kind="ExternalInput").ap()
    b1_d = nc.dram_tensor("b1r", [2, CH], f32, kind="ExternalInput").ap()
    wr_d = nc.dram_tensor("wrs", [CH, NE], f32, kind="ExternalInput").ap()
    br_d = nc.dram_tensor("brr", [1, NE], f32, kind="ExternalInput").ap()
    ws_d = nc.dram_tensor("wst", [CH, 9 * CH], dmm, kind="ExternalInput").ap()
    bs_d = nc.dram_tensor("bsr", [CH, 1], f32, kind="ExternalInput").ap()
    we_d = nc.dram_tensor("wet", [NE, CH, 9 * CH], f32, kind="ExternalInput").ap()
    be_d = nc.dram_tensor("ber", [CH, NE], f32, kind="ExternalInput").ap()
    w2_d = nc.dram_tensor("w2t", [3, CH, C2], dmm, kind="ExternalInput").ap()
    b2_d = nc.dram_tensor("b2r", [2, CH], f32, kind="ExternalInput").ap()
    y_d = nc.dram_tensor(
        "y", [BPC, 2, CH, NPIX], f32,
        kind="Internal" if internal_io else "ExternalOutput").ap()

    wpool = ctx.enter_context(tc.tile_pool(name="weights", bufs=1))
    ppool = ctx.enter_context(tc.tile_pool(name="persist", bufs=1))
    xpool = ctx.enter_context(tc.tile_pool(name="xin", bufs=tune["xbufs"]))
    opool = ctx.enter_context(tc.tile_pool(name="oout", bufs=tune["obufs"]))
    rpool = ctx.enter_context(tc.tile_pool(name="rtile", bufs=tune["rbufs"]))
    spool = ctx.enter_context(tc.tile_pool(name="small", bufs=2))
    selpool = ctx.enter_context(tc.tile_pool(name="sel", bufs=1))
    psum = ctx.enter_context(tc.tile_pool(name="psum", bufs=tune["psbufs"], space="PSUM"))
    psumS = ctx.enter_context(tc.tile_pool(name="psumS", bufs=1, space="PSUM"))

    # ---- load weights into SBUF (resident) ----
    w1_sb = wpool.tile([CH, 2 * 2 * CH], dmm)
    for k in range(2):
        nc.sync.dma_start(w1_sb[:, k * 256:(k + 1) * 256], w1_d[k])
    ws_sb = wpool.tile([CH, 9 * CH], dmm)
    nc.sync.dma_start(ws_sb[:], ws_d)
    we_sb = wpool.tile([CH, NE * 9 * CH], f32)
    for e in range(NE):
        nc.sync.dma_start(we_sb[:, e * 1152:(e + 1) * 1152], we_d[e])
    w2_sb = wpool.tile([CH, 3 * C2], dmm)
    for k in range(3):
        nc.sync.dma_start(w2_sb[:, k * 256:(k + 1) * 256], w2_d[k])
    wr_sb = wpool.tile([CH, NE], f32)
    nc.sync.dma_start(wr_sb[:], wr_d)
    br_sb = wpool.tile([1, NE], f32)
    nc.sync.dma_start(br_sb[:], br_d)
    bs_sb = wpool.tile([CH, 1], f32)
    nc.sync.dma_start(bs_sb[:], bs_d)
    be_sb = wpool.tile([CH, NE], f32)
    nc.sync.dma_start(be_sb[:], be_d)
    b1_sb = wpool.tile([CH, 2], f32)
    for k in range(2):
        nc.sync.dma_start(b1_sb[:, k:k + 1], b1_d[k])
    b2_sb = wpool.tile([CH, 2], f32)
    for k in range(2):
        nc.sync.dma_start(b2_sb[:, k:k + 1], b2_d[k])
    ones_sb = wpool.tile([1, CH], f32)
    nc.vector.memset(ones_sb[:], 1.0)

    if internal_io:
        # timing mode: x is Internal (uninitialized) DRAM; zero it once so
        # the timed loop computes on deterministic, non-denormal data.
        zs = wpool.tile([CH, 800], dmm, name="zs")
        if tune["bf16"]:
            nc.vector.memset(zs[:], 0.0)
        else:
            nc.vector.memset(zs[:].bitcast(f32), 0.0)
        for zb in range(BPC):
            for zk in range(2):
                for zj in range(NPIX // 800):
                    nc.sync.dma_start(
                        x_d[zb, zk, :, zj * 800:(zj + 1) * 800], zs[:])

    # ---- persistent per-sample working buffers ----
    # (optionally double-buffered across samples to decouple next-sample cv1
    # writes from current-sample conv/cv2 reads)
    fps = []
    for fi in range(2 if tune["fpdouble"] else 1):
        fp = ppool.tile([CH, PADH * PADW], dmm, tag=f"fp{fi}", name=f"fp{fi}")
        # zero once: borders stay zero forever (bitcast: memset lacks f32r)
        if tune["bf16"]:
            nc.vector.memset(fp[:], 0.0)
        else:
            nc.vector.memset(fp[:].bitcast(f32), 0.0)
        fps.append(fp[:].rearrange("p (r c) -> p r c", c=PADW))
    a_sbs = [ppool.tile([CH, NPIX], dmm, tag=f"a{ai}", name=f"a{ai}")
             for ai in range(2 if tune["adouble"] else 1)]
    sh_sb = ppool.tile([CH, NPIX], bf16)
    moe_sb = ppool.tile([CH, NPIX], dmm)
    # fixed-address x tiles for (b=0, pi=0..1): filled in a prologue before
    # the rep loop and re-filled at the END of each body iteration, so the
    # next rep's cv1 never waits behind this rep's y-store DMA queue.
    xt_pre = []
    for pp in range(2):
        t0 = ppool.tile([CH, 2 * TN], dmm, tag=f"xtp0{pp}", name=f"xtp0{pp}")
        t1 = ppool.tile([CH, 2 * TN], dmm, tag=f"xtp1{pp}", name=f"xtp1{pp}")
        nc.sync.dma_start(t0[:], x_d[0, 0, :, pp * 800:(pp + 1) * 800])
        nc.sync.dma_start(t1[:], x_d[0, 1, :, pp * 800:(pp + 1) * 800])
        xt_pre.append((t0, t1))

    tmpool = ctx.enter_context(tc.tile_pool(name="silutmp", bufs=2)) if sim_compat else None

    def act_silu(out_ap, ps_ap, bias_ap, accum_ap=None):
        """SiLU from PSUM -> SBUF. On HW, one ACT instruction (with optional
        free GAP accumulation). CoreSim lacks Silu, so sim_compat emulates via
        Sigmoid + (ps+bias)*sig, and computes the accumulation separately."""
        if not sim_compat:
            if accum_ap is not None:
                nc.scalar.activation(out_ap, ps_ap, AF.Silu, bias=bias_ap,
                                     scale=1.0, accum_out=accum_ap)
            else:
                nc.scalar.activation(out_ap, ps_ap, AF.Silu, bias=bias_ap,
                                     scale=1.0)
            return
        shp = list(out_ap.shape[1:])
        fs = 1
        for d in shp:
            fs *= d
        tmp = tmpool.tile([CH, 2 * TN], f32, tag="sigmoid_tmp")
        tv = tmp[:, 0:fs]
        if len(shp) == 2:
            tv = tv.rearrange("p (g c) -> p g c", g=shp[0])
        elif len(shp) == 3:
            tv = tv.rearrange("p (g r c) -> p g r c", g=shp[0], r=shp[1])
        nc.scalar.activation(tv, ps_ap, AF.Sigmoid, bias=bias_ap, scale=1.0)
        nc.vector.scalar_tensor_tensor(out_ap, ps_ap, bias_ap, tv,
                                       op0=OP.add, op1=OP.mult)
        if accum_ap is not None:
            axis = [None, AX.X, AX.XY, AX.XYZ][len(shp)]
            nc.vector.reduce_sum(accum_ap, out_ap, axis=axis)

    def conv_tile_matmuls(ps, wsb, i, fp3):
        for t, (dy, dx) in enumerate(TAPS):
            rhs = fp3[:, i * RPT + dy: i * RPT + dy + RPT, dx: dx + W]
            nc.tensor.matmul(
                ps[:],
                wsb[:, t * CH:(t + 1) * CH],
                rhs,
                start=(t == 0),
                stop=(t == 8),
            )

    def _body():
        xt_cache = {}

        def emit_xt(b, pi):
            xt0 = xpool.tile([CH, 2 * TN], dmm, tag="xt0")
            nc.sync.dma_start(xt0[:], x_d[b, 0, :, pi * 800:(pi + 1) * 800])
            xt1 = xpool.tile([CH, 2 * TN], dmm, tag="xt1")
            nc.sync.dma_start(xt1[:], x_d[b, 1, :, pi * 800:(pi + 1) * 800])
            return xt0, xt1

        for b in range(BPC):
            fp3 = fps[b % len(fps)]
            fp3v = fp3  # [128, 82, 82] padded view
            a_sb = a_sbs[b % len(a_sbs)]
            # ---- cv1 over tile-PAIRS: 800 px per ACT, shared-weight MM runs,
            # GAP accumulated for free ----
            gap_sb = spool.tile([CH, NP], f32, tag="gap")
            for pi in range(NP):
                i0 = 2 * pi
                if b == 0 and pi < 2:
                    xt0, xt1 = xt_pre[pi]
                elif (b, pi) in xt_cache:
                    xt0, xt1 = xt_cache.pop((b, pi))
                else:
                    xt0, xt1 = emit_xt(b, pi)
                ps_a = psum.tile([CH, 2, 512], f32, tag="ps")
                ps_f = psum.tile([CH, 2, 512], f32, tag="ps")
                for k, xt in ((0, xt0), (1, xt1)):
                    for hw_, ps2 in ((0, ps_a), (1, ps_f)):
                        wsl = w1_sb[:, k * 256 + hw_ * 128: k * 256 + hw_ * 128 + 128]
                        for ii in range(2):
                            nc.tensor.matmul(ps2[:, ii, 0:TN], wsl,
                                             xt[:, ii * TN:(ii + 1) * TN],
                                             start=(k == 0), stop=(k == 1))
                act_silu(a_sb[:, i0 * TN:(i0 + 2) * TN].rearrange(
                             "p (g c) -> p g c", g=2),
                         ps_a[:, :, 0:TN], b1_sb[:, 0:1])
                fout = fp3v[:, 1 + 10 * pi: 11 + 10 * pi, 1:1 + W].rearrange(
                    "p (g r) c -> p g r c", g=2)
                act_silu(fout,
                         ps_f[:, :, 0:TN].rearrange("p g (r c) -> p g r c", c=W),
                         b1_sb[:, 1:2], accum_ap=gap_sb[:, pi:pi + 1])

            # ---- router: logits -> softmax -> top-1 one-hot + gate ----
            pooled = spool.tile([CH, 1], f32, tag="pooled")
            nc.vector.reduce_sum(pooled[:], gap_sb[:], axis=AX.X)
            ps_l = psumS.tile([1, NE], f32, tag="psl")
            # wr is pre-scaled by 1/NPIX on the host, so sums (not means) work.
            nc.tensor.matmul(ps_l[:], pooled[:], wr_sb[:], start=True, stop=True)
            logits = spool.tile([1, NE], f32, tag="logits")
            nc.vector.tensor_add(logits[:], ps_l[:], br_sb[:])
            m_sb = spool.tile([1, 1], f32, tag="m")
            nc.vector.reduce_max(m_sb[:], logits[:], axis=AX.X)
            negm = spool.tile([1, 1], f32, tag="negm")
            nc.vector.tensor_scalar_mul(negm[:], m_sb[:], -0.5)
            t_sb = spool.tile([1, NE], f32, tag="tsb")
            nc.scalar.activation(t_sb[:], logits[:], AF.Tanh, bias=negm[:],
                                 scale=0.5)
            num = spool.tile([1, NE], f32, tag="num")
            nc.vector.tensor_scalar_add(num[:], t_sb[:], 1.0)
            den = spool.tile([1, NE], f32, tag="den")
            nc.vector.tensor_scalar(den[:], t_sb[:], -1.0, 1.0,
                                    op0=OP.mult, op1=OP.add)
            rden = spool.tile([1, NE], f32, tag="rden")
            nc.vector.reciprocal(rden[:], den[:])
            e_sb = spool.tile([1, NE], f32, tag="esb")
            nc.vector.tensor_mul(e_sb[:], num[:], rden[:])
            s_sb = spool.tile([1, 1], f32, tag="ssb")
            nc.vector.reduce_sum(s_sb[:], e_sb[:], axis=AX.X)
            wgt = spool.tile([1, 1], f32, tag="wgt")
            nc.vector.reciprocal(wgt[:], s_sb[:])
            oh = spool.tile([1, NE], f32, tag="oh")
            nc.vector.tensor_scalar(oh[:], logits[:], m_sb[:], None, op0=OP.is_ge)
            bc = spool.tile([1, NE + 1], f32, tag="bc")
            nc.vector.tensor_copy(bc[:, 0:NE], oh[:])
            nc.vector.tensor_copy(bc[:, NE:NE + 1], wgt[:])
            ps_bc = psumS.tile([CH, NE + 1], f32, tag="psb")
            nc.tensor.matmul(ps_bc[:], ones_sb[:], bc[:], start=True, stop=True)
            sc = spool.tile([CH, NE + 1], f32, tag="sc")
            nc.vector.tensor_copy(sc[:], ps_bc[:])

            # ---- expert-weight select: Wsel = sum_e onehot[e] * We[e] ----
            wA = selpool.tile([CH, 9 * CH], f32, tag="wA")
            nc.vector.tensor_scalar_mul(wA[:], we_sb[:, 0:1152], sc[:, 0:1])
            wB = selpool.tile([CH, 9 * CH], f32, tag="wB")
            nc.vector.scalar_tensor_tensor(wB[:], we_sb[:, 1152:2304], sc[:, 1:2],
                                           wA[:], op0=OP.mult, op1=OP.add)
            wS = selpool.tile([CH, 9 * CH], dmm, tag="wS")
            nc.vector.scalar_tensor_tensor(wS[:], we_sb[:, 2304:3456], sc[:, 2:3],
                                           wB[:], op0=OP.mult, op1=OP.add)
            bA = spool.tile([CH, 1], f32, tag="bA")
            nc.vector.tensor_scalar_mul(bA[:], be_sb[:, 0:1], sc[:, 0:1])
            bB = spool.tile([CH, 1], f32, tag="bB")
            nc.vector.scalar_tensor_tensor(bB[:], be_sb[:, 1:2], sc[:, 1:2],
                                           bA[:], op0=OP.mult, op1=OP.add)
            bS = spool.tile([CH, 1], f32, tag="bS")
            nc.vector.scalar_tensor_tensor(bS[:], be_sb[:, 2:3], sc[:, 2:3],
                                           bB[:], op0=OP.mult, op1=OP.add)

            # prefetch next sample's first x pairs ahead of this sample's
            # y stores in the DMA queue (slots are free: cv1(b) is done)
            if b + 1 < BPC:
                for pp in range(2):
                    xt_cache[(b + 1, pp)] = emit_xt(b + 1, pp)
            else:
                # refill the loop-carried preload tiles for the next rep's
                # first sample (reads of this rep's b=0 are long done)
                for pp in range(2):
                    t0, t1 = xt_pre[pp]
                    nc.sync.dma_start(t0[:], x_d[0, 0, :, pp * 800:(pp + 1) * 800])
                    nc.sync.dma_start(t1[:], x_d[0, 1, :, pp * 800:(pp + 1) * 800])

            def conv_pair(ps2, wsb, pi):
                i0 = 2 * pi
                for t, (dy, dx) in enumerate(TAPS):
                    wt = wsb[:, t * CH:(t + 1) * CH]
                    for ii in range(2):
                        rhs = fp3[:, (i0 + ii) * RPT + dy: (i0 + ii) * RPT + dy + RPT,
                                  dx: dx + W]
                        nc.tensor.matmul(ps2[:, ii, 0:TN], wt, rhs,
                                         start=(t == 0), stop=(t == 8))

            # ---- shared expert 3x3 conv + SiLU (pairs) ----
            for pi in range(NP):
                ps2 = psum.tile([CH, 2, 512], f32, tag="ps")
                conv_pair(ps2, ws_sb, pi)
                act_silu(sh_sb[:, pi * 800:(pi + 1) * 800].rearrange(
                             "p (g c) -> p g c", g=2),
                         ps2[:, :, 0:TN], bs_sb[:])

            # ---- routed conv + moe + fused cv2, software-pipelined by 1 pair ----
            def cv2_pair(pi):
                i0 = 2 * pi
                for h in range(2):
                    po = psum.tile([CH, 2, 512], f32, tag="ps")
                    for c, src_of in ((0, None), (1, None), (2, None)):
                        wsl = w2_sb[:, c * 256 + h * 128: c * 256 + h * 128 + 128]
                        for ii in range(2):
                            i = i0 + ii
                            if c == 0:
                                rhs = a_sb[:, i * TN:(i + 1) * TN]
                            elif c == 1:
                                rhs = fp3[:, i * RPT + 1: i * RPT + 1 + RPT,
                                          1: 1 + W]
                            else:
                                rhs = moe_sb[:, i * TN:(i + 1) * TN]
                            nc.tensor.matmul(po[:, ii, 0:TN], wsl, rhs,
                                             start=(c == 0), stop=(c == 2))
                    ot = opool.tile([CH, 2 * TN], f32, tag="ot")
                    act_silu(ot[:].rearrange("p (g c) -> p g c", g=2),
                             po[:, :, 0:TN], b2_sb[:, h:h + 1])
                    nc.sync.dma_start(y_d[b, h, :, pi * 800:(pi + 1) * 800], ot[:])

            for pi in range(NP):
                ps2 = psum.tile([CH, 2, 512], f32, tag="ps")
                conv_pair(ps2, wS, pi)
                rt = rpool.tile([CH, 2 * TN], bf16, tag="rt")
                act_silu(rt[:].rearrange("p (g c) -> p g c", g=2),
                         ps2[:, :, 0:TN], bS[:])
                nc.vector.scalar_tensor_tensor(
                    moe_sb[:, pi * 800:(pi + 1) * 800], rt[:], sc[:, NE:NE + 1],
                    sh_sb[:, pi * 800:(pi + 1) * 800], op0=OP.mult, op1=OP.add)
                if pi > 0:
                    cv2_pair(pi - 1)
            cv2_pair(NP - 1)

    if reps == 1:
        _body()
    else:
        # HW timing mode: repeat the whole workload in a hardware loop
        # (same instruction count / compile cost; R x device work).
        with tc.For_i(0, reps, 1):
            _body()
    if internal_io:
        # tiny external output so the (otherwise internal-IO) program is not
        # dead-code eliminated; depends on the looped work via y.
        ydig_d = nc.dram_tensor("ydig", [CH, 4], f32,
                                kind="ExternalOutput").ap()
        ydig_t = opool.tile([CH, 4], f32, name="ydig_t")
        nc.sync.dma_start(ydig_t[:], y_d[0, 0, :, 0:4])
        nc.sync.dma_start(ydig_d, ydig_t[:])


def _ldw_key(ins):
    w = ins.ins[0]
    return (str(getattr(w, "memref", None)), str(getattr(w, "ap", None)),
            getattr(w, "offset", None), str(getattr(w, "dtype", None)),
            str(ins.perf_mode), str(ins.is_transpose))


def dedupe_ldweights(nc):
    """Drop InstLdweights that reload the identical weights as the previous
    Ldweights in the block (adjacent L-M pattern only); move their
    waits/updates onto the immediately-following matmul. The PE keeps the
    stationary operand across matmuls, so the reload is pure overhead."""
    ndrop = 0
    for blk in nc.main_func.blocks:
        out = []
        last_key = None
        pend = None  # dropped ldweights whose sync must move to next matmul
        for ins in blk.instructions:
            if isinstance(ins, mybir.InstLdweights):
                key = _ldw_key(ins)
                if key == last_key and pend is None:
                    pend = ins
                    ndrop += 1
                    continue
                last_key = key
            elif isinstance(ins, mybir.InstMatmult):
                if pend is not None:
                    si, pi = ins.sync_info, pend.sync_info
                    if pi is not None and si is not None:
                        for w in list(pi.on_wait or []):
                            si.on_wait.append(w)
                        for u in list(pi.on_update or []):
                            si.on_update.append(u)
                    pend = None
                if ins.ldweights is not False:
                    last_key = None  # self-loading matmul changes PE weights
            elif pend is not None:
                # something else between the dropped L and its M: bail out,
                # restore the load to stay safe.
                out.append(pend)
                pend = None
                ndrop -= 1
            out.append(ins)
        assert pend is None
        blk.instructions[:] = out
    return ndrop


def build(reps=1, sim_compat=False, tune=None, internal_io=False):
    from contextlib import ExitStack
    nc = bacc.Bacc("TRN2", target_bir_lowering=False, debug=False,
                   num_devices=NCORES)
    with tile.TileContext(nc) as tc:
        with ExitStack() as ctx:
            _emit(nc, tc, ctx, reps=reps, sim_compat=sim_compat, tune=tune,
                  internal_io=internal_io)
    nc.compile()
    dedupe_ldweights(nc)
    return nc


def round_f32r(a):
    """Round fp32 to the PE's fp32r format: 11 explicit mantissa bits
    (round-to-nearest-even), low 12 bits zero. The result is both a valid
    fp32 value and a valid fp32r bit pattern."""
    a = np.ascontiguousarray(np.asarray(a, np.float32))
    bits = a.view(np.uint32).astype(np.uint64)
    lsb = (bits >> 12) & 1
    r = (bits + 0x7FF + lsb) & 0xFFFFF000
    return r.astype(np.uint32).view(np.float32)


def marshal_inputs(x, w1, b1, wr, br, ws, bs, we, be, w2, b2, use_bf16=True):
    """Host-side (tiny) weight re-layouts into matmul-friendly forms."""
    asf = lambda a: np.ascontiguousarray(np.asarray(a, dtype=np.float32))
    if use_bf16:
        import ml_dtypes
        cvt = lambda a: np.ascontiguousarray(
            np.asarray(a, np.float32).astype(ml_dtypes.bfloat16))
    else:
        cvt = round_f32r
    x = cvt(x)
    w1t = asf(np.asarray(w1, np.float32).reshape(2 * CH, C1).T.reshape(2, CH, 2 * CH))
    b1r = asf(np.asarray(b1, np.float32).reshape(2, CH))
    wrs = asf(np.asarray(wr, np.float32) / NPIX)
    brr = asf(np.asarray(br, np.float32).reshape(1, NE))
    wst = asf(np.asarray(ws, np.float32).transpose(1, 2, 3, 0).reshape(CH, 9 * CH))
    bsr = asf(np.asarray(bs, np.float32).reshape(CH, 1))
    wet = asf(np.asarray(we, np.float32).transpose(0, 2, 3, 4, 1).reshape(NE, CH, 9 * CH))
    ber = asf(np.asarray(be, np.float32).T)
    w2t = asf(np.asarray(w2, np.float32).reshape(C2, 3 * CH).T.reshape(3, CH, C2))
    b2r = asf(np.asarray(b2, np.float32).reshape(2, CH))
    w1t = cvt(w1t)
    wst = cvt(wst)
    wet = round_f32r(wet) if not use_bf16 else wet
    w2t = cvt(w2t)
    shared = dict(w1t=w1t, b1r=b1r, wrs=wrs, brr=brr, wst=wst, bsr=bsr,
                  wet=wet, ber=ber, w2t=w2t, b2r=b2r)
    xc = x.reshape(NCORES, BPC, 2, CH, NPIX)
    in_maps = [dict(shared, x=np.ascontiguousarray(xc[c])) for c in range(NCORES)]
    return in_maps


_CACHE = {}


def _get_nc():
    if "nc" not in _CACHE:
        _CACHE["nc"] = build(reps=1)
    return _CACHE["nc"]


def _get_runner():
    """Build the sharded PJRT callable once (mirrors
    bass2jax.run_bass_via_pjrt's multi-core path) so repeat kernel() calls
    skip the jax retrace/compile."""
    if "runner" in _CACHE:
        return _CACHE["runner"]
    import jax
    from jax.experimental.shard_map import shard_map
    from jax.sharding import Mesh, PartitionSpec
    from concourse import bass2jax

    nc = _get_nc()
    bass2jax.install_neuronx_cc_hook()
    part_name = nc.partition_id_tensor.name if nc.partition_id_tensor else None
    in_names, out_names, out_avals = [], [], []
    for alloc in nc.m.functions[0].allocations:
        if not isinstance(alloc, mybir.MemoryLocationSet):
            continue
        name = alloc.memorylocations[0].name
        if alloc.kind == "ExternalInput":
            if name != part_name:
                in_names.append(name)
        elif alloc.kind == "ExternalOutput":
            out_names.append(name)
            out_avals.append(jax.core.ShapedArray(
                tuple(alloc.tensor_shape), mybir.dt.np(alloc.dtype)))
    assert nc.dbg_addr is None
    n_params = len(in_names)
    all_in = in_names + out_names  # zero buffers donated as outputs
    if part_name is not None:
        all_in = all_in + [part_name]

    def _body(*args):
        operands = list(args)
        if part_name is not None:
            operands.append(bass2jax.partition_id_tensor())
        outs = bass2jax._bass_exec_p.bind(
            *operands, out_avals=tuple(out_avals), in_names=tuple(all_in),
            out_names=tuple(out_names), lowering_input_output_aliases=(),
            sim_require_finite=True, sim_require_nnan=True, nc=nc)
        return tuple(outs)

    devices = jax.devices()[:NCORES]
    mesh = Mesh(np.asarray(devices), ("core",))
    nio = n_params + len(out_names)
    sharded = jax.jit(
        shard_map(_body, mesh=mesh, in_specs=(PartitionSpec("core"),) * nio,
                  out_specs=(PartitionSpec("core"),) * len(out_names),
                  check_rep=False),
        donate_argnums=tuple(range(n_params, nio)), keep_unused=True)
    _CACHE["runner"] = (sharded, in_names, out_names, out_avals)
    return _CACHE["runner"]


def kernel(x, w1, b1, wr, br, ws, bs, we, be, w2, b2):
    in_maps = marshal_inputs(x, w1, b1, wr, br, ws, bs, we, be, w2, b2)
    sharded, in_names, out_names, out_avals = _get_runner()
    concat_in = [
        np.concatenate([in_maps[c][name] for c in range(NCORES)], axis=0)
        for name in in_names
    ]
    concat_zeros = [
        np.zeros((NCORES * a.shape[0], *a.shape[1:]), a.dtype) for a in out_avals
    ]
    out_arrs = sharded(*concat_in, *concat_zeros)
    y = np.asarray(out_arrs[out_names.index("y")])
    return np.ascontiguousarray(y.reshape(B, C2, H, W))



# revision 17
# speedup vs baseline: 1.2092x; 1.0216x over previous
"""Trainium2 Bass kernel for nn_C2f_DualModal_MoE (C2f block with top-1 MoE routing).

Strategy (data-parallel over batch, 4 samples per core on 8 cores):
  - all matmuls in bf16 (same PE rate as f32r but ~60 fewer overhead cycles
    per matmul: FWL fast-weight-load triggers for non-fp32 dtypes), with
    redundant LDWEIGHTS for consecutive same-weight matmuls dropped
    post-compile (each conv tap / cv2 chunk loads once per ii-pair);
  - cv1 (1x1 conv 256->256 + SiLU) as bf16 matmuls over 400-pixel tiles;
    the `feat` half is written into a zero-padded [82x82] spatial layout so
    the 3x3 convs become 9 shift-offset matmuls. The global-average-pool for
    the router comes free via the activation accum_out.
  - Router: tiny f32 matmul + softmax on-chip; the top-1 selection is turned
    into a one-hot vector (no control flow), which selects the routed expert's
    weights via 3 vector ops (Wsel = sum_e onehot[e] * We[e]); since top-1,
    conv(feat, Wsel) == conv(feat, We[argmax]).
  - shared + routed 3x3 convs (SiLU), moe = shared + gate * routed.
  - cv2 (1x1 conv 384->256 + SiLU) fused per tile from (a, feat, moe) without
    materializing the concat (chunk-outer order for weight reuse); routed-conv
    and cv2 are software-pipelined by one tile.
  - router softmax uses tanh ((1+t)/(1-t) identity) instead of exp so the ACT
    engine never swaps its activation table away from the Silu set.
  - x tiles for the next sample are prefetched ahead of this sample's y-store
    DMAs; sample 0's first pairs live in loop-carried fixed tiles refilled at
    the end of each rep so the hardware rep loop restarts without DMA waits.
Matmuls bf16, accumulation f32; rel err ~4.0e-3 vs the 2e-2 gate.
"""

import numpy as np

import concourse.bass as bass
import concourse.bacc as bacc
import concourse.tile as tile
from concourse import mybir
from concourse.bass_utils import run_bass_kernel_spmd

# Problem constants (hardcoded per contract)
B, C1, C2 = 32, 256, 256
H = W = 80
CH = 128
NE = 3
NCORES = 8
BPC = B // NCORES          # samples per core = 4
NPIX = H * W               # 6400
PADW = W + 2               # 82
PADH = H + 2               # 82
RPT = 5                    # rows per pixel tile
TN = RPT * W               # 400 pixels per tile
NT = H // RPT              # 16 tiles
NP = NT // 2               # 8 tile-pairs
TAPS = [(dy, dx) for dy in range(3) for dx in range(3)]

f32 = mybir.dt.float32
f32r = mybir.dt.float32r
bf16 = mybir.dt.bfloat16


def _emit(nc, tc, ctx, reps=1, sim_compat=False, tune=None, internal_io=False):
    AX = mybir.AxisListType
    OP = mybir.AluOpType
    AF = mybir.ActivationFunctionType
    tune = {**dict(xbufs=4, obufs=4, rbufs=2, psbufs=3, fpdouble=True,
                   adouble=True, bf16=True), **(tune or {})}
    dmm = bf16 if tune["bf16"] else f32r

    io_kind = "Internal" if internal_io else "ExternalInput"
    x_d = nc.dram_tensor("x", [BPC, 2, CH, NPIX], dmm, kind=io_kind).ap()
    w1_d = nc.dram_tensor("w1t", [2, CH, 2 * CH], dmm, kind="ExternalInput").ap()
    b1_d = nc.dram_tensor("b1r", [2, CH], f32, kind="ExternalInput").ap()
    wr_d = nc.dram_tensor("wrs", [CH, NE], f32, kind="ExternalInput").ap()
    br_d = nc.dram_tensor("brr", [1, NE], f32, kind="ExternalInput").ap()
    ws_d = nc.dram_tensor("wst", [CH, 9 * CH], dmm, kind="ExternalInput").ap()
    bs_d = nc.dram_tensor("bsr", [CH, 1], f32, kind="ExternalInput").ap()
    we_d = nc.dram_tensor("wet", [NE, CH, 9 * CH], f32, kind="ExternalInput").ap()
    be_d = nc.dram_tensor("ber", [CH, NE], f32, kind="ExternalInput").ap()
    w2_d = nc.dram_tensor("w2t", [3, CH, C2], dmm, kind="ExternalInput").ap()
    b2_d = nc.dram_tensor("b2r", [2, CH], f32, kind="ExternalInput").ap()
    y_d = nc.dram_tensor(
        "y", [BPC, 2, CH, NPIX], f32,
        kind="Internal" if internal_io else "ExternalOutput").ap()

    wpool = ctx.enter_context(tc.tile_pool(name="weights", bufs=1))
    ppool = ctx.enter_context(tc.tile_pool(name="persist", bufs=1))
    xpool = ctx.enter_context(tc.tile_pool(name="xin", bufs=tune["xbufs"]))
    opool = ctx.enter_context(tc.tile_pool(name="oout", bufs=tune["obufs"]))
    rpool = ctx.enter_context(tc.tile_pool(name="rtile", bufs=tune["rbufs"]))
    spool = ctx.enter_context(tc.tile_pool(name="small", bufs=2))
    selpool = ctx.enter_context(tc.tile_pool(name="sel", bufs=1))
    psum = ctx.enter_context(tc.tile_pool(name="psum", bufs=tune["psbufs"], space="PSUM"))
    psumS = ctx.enter_context(tc.tile_pool(name="psumS", bufs=1, space="PSUM"))

    # ---- load weights into SBUF (resident) ----
    w1_sb = wpool.tile([CH, 2 * 2 * CH], dmm)
    for k in range(2):
        nc.sync.dma_start(w1_sb[:, k * 256:(k + 1) * 256], w1_d[k])
    ws_sb = wpool.tile([CH, 9 * CH], dmm)
    nc.sync.dma_start(ws_sb[:], ws_d)
    we_sb = wpool.tile([CH, NE * 9 * CH], f32)
    for e in range(NE):
        nc.sync.dma_start(we_sb[:, e * 1152:(e + 1) * 1152], we_d[e])
    w2_sb = wpool.tile([CH, 3 * C2], dmm)
    for k in range(3):
        nc.sync.dma_start(w2_sb[:, k * 256:(k + 1) * 256], w2_d[k])
    wr_sb = wpool.tile([CH, NE], f32)
    nc.sync.dma_start(wr_sb[:], wr_d)
    br_sb = wpool.tile([1, NE], f32)
    nc.sync.dma_start(br_sb[:], br_d)
    bs_sb = wpool.tile([CH, 1], f32)
    nc.sync.dma_start(bs_sb[:], bs_d)
    be_sb = wpool.tile([CH, NE], f32)
    nc.sync.dma_start(be_sb[:], be_d)
    b1_sb = wpool.tile([CH, 2], f32)
    for k in range(2):
        nc.sync.dma_start(b1_sb[:, k:k + 1], b1_d[k])
    b2_sb = wpool.tile([CH, 2], f32)
    for k in range(2):
        nc.sync.dma_start(b2_sb[:, k:k + 1], b2_d[k])
    ones_sb = wpool.tile([1, CH], f32)
    nc.vector.memset(ones_sb[:], 1.0)

    if internal_io:
        # timing mode: x is Internal (uninitialized) DRAM; zero it once so
        # the timed loop computes on deterministic, non-denormal data.
        zs = wpool.tile([CH, 800], dmm, name="zs")
        if tune["bf16"]:
            nc.vector.memset(zs[:], 0.0)
        else:
            nc.vector.memset(zs[:].bitcast(f32), 0.0)
        for zb in range(BPC):
            for zk in range(2):
                for zj in range(NPIX // 800):
                    nc.sync.dma_start(
                        x_d[zb, zk, :, zj * 800:(zj + 1) * 800], zs[:])

    # ---- persistent per-sample working buffers ----
    # (optionally double-buffered across samples to decouple next-sample cv1
    # writes from current-sample conv/cv2 reads)
    fps = []
    for fi in range(2 if tune["fpdouble"] else 1):
        fp = ppool.tile([CH, PADH * PADW], dmm, tag=f"fp{fi}", name=f"fp{fi}")
        # zero once: borders stay zero forever (bitcast: memset lacks f32r)
        if tune["bf16"]:
            nc.vector.memset(fp[:], 0.0)
        else:
            nc.vector.memset(fp[:].bitcast(f32), 0.0)
        fps.append(fp[:].rearrange("p (r c) -> p r c", c=PADW))
    a_sbs = [ppool.tile([CH, NPIX], dmm, tag=f"a{ai}", name=f"a{ai}")
             for ai in range(2 if tune["adouble"] else 1)]
    sh_sb = ppool.tile([CH, NPIX], bf16)
    moe_sb = ppool.tile([CH, NPIX], dmm)
    # fixed-address x tiles for (b=0, pi=0..1): filled in a prologue before
    # the rep loop and re-filled at the END of each body iteration, so the
    # next rep's cv1 never waits behind this rep's y-store DMA queue.
    xt_pre = []
    for pp in range(2):
        t0 = ppool.tile([CH, 2 * TN], dmm, tag=f"xtp0{pp}", name=f"xtp0{pp}")
        t1 = ppool.tile([CH, 2 * TN], dmm, tag=f"xtp1{pp}", name=f"xtp1{pp}")
        nc.sync.dma_start(t0[:], x_d[0, 0, :, pp * 800:(pp + 1) * 800])
        nc.sync.dma_start(t1[:], x_d[0, 1, :, pp * 800:(pp + 1) * 800])
        xt_pre.append((t0, t1))

    tmpool = ctx.enter_context(tc.tile_pool(name="silutmp", bufs=2)) if sim_compat else None

    def act_silu(out_ap, ps_ap, bias_ap, accum_ap=None):
        """SiLU from PSUM -> SBUF. On HW, one ACT instruction (with optional
        free GAP accumulation). CoreSim lacks Silu, so sim_compat emulates via
        Sigmoid + (ps+bias)*sig, and computes the accumulation separately."""
        if not sim_compat:
            if accum_ap is not None:
                nc.scalar.activation(out_ap, ps_ap, AF.Silu, bias=bias_ap,
                                     scale=1.0, accum_out=accum_ap)
            else:
                nc.scalar.activation(out_ap, ps_ap, AF.Silu, bias=bias_ap,
                                     scale=1.0)
            return
        shp = list(out_ap.shape[1:])
        fs = 1
        for d in shp:
            fs *= d
        tmp = tmpool.tile([CH, 2 * TN], f32, tag="sigmoid_tmp")
        tv = tmp[:, 0:fs]
        if len(shp) == 2:
            tv = tv.rearrange("p (g c) -> p g c", g=shp[0])
        elif len(shp) == 3:
            tv = tv.rearrange("p (g r c) -> p g r c", g=shp[0], r=shp[1])
        nc.scalar.activation(tv, ps_ap, AF.Sigmoid, bias=bias_ap, scale=1.0)
        nc.vector.scalar_tensor_tensor(out_ap, ps_ap, bias_ap, tv,
                                       op0=OP.add, op1=OP.mult)
        if accum_ap is not None:
            axis = [None, AX.X, AX.XY, AX.XYZ][len(shp)]
            nc.vector.reduce_sum(accum_ap, out_ap, axis=axis)

    def conv_tile_matmuls(ps, wsb, i, fp3):
        for t, (dy, dx) in enumerate(TAPS):
            rhs = fp3[:, i * RPT + dy: i * RPT + dy + RPT, dx: dx + W]
            nc.tensor.matmul(
                ps[:],
                wsb[:, t * CH:(t + 1) * CH],
                rhs,
                start=(t == 0),
                stop=(t == 8),
            )

    def _body():
        xt_cache = {}

        def emit_xt(b, pi):
            xt0 = xpool.tile([CH, 2 * TN], dmm, tag="xt0")
            nc.sync.dma_start(xt0[:], x_d[b, 0, :, pi * 800:(pi + 1) * 800])
            xt1 = xpool.tile([CH, 2 * TN], dmm, tag="xt1")
            nc.sync.dma_start(xt1[:], x_d[b, 1, :, pi * 800:(pi + 1) * 800])
            return xt0, xt1

        for b in range(BPC):
            fp3 = fps[b % len(fps)]
            fp3v = fp3  # [128, 82, 82] padded view
            a_sb = a_sbs[b % len(a_sbs)]
            # ---- cv1 over tile-PAIRS: 800 px per ACT, shared-weight MM runs,
            # GAP accumulated for free ----
            gap_sb = spool.tile([CH, NP], f32, tag="gap")
            for pi in range(NP):
                i0 = 2 * pi
                if b == 0 and pi < 2:
                    xt0, xt1 = xt_pre[pi]
                elif (b, pi) in xt_cache:
                    xt0, xt1 = xt_cache.pop((b, pi))
                else:
                    xt0, xt1 = emit_xt(b, pi)
                ps_a = psum.tile([CH, 2, 512], f32, tag="ps")
                ps_f = psum.tile([CH, 2, 512], f32, tag="ps")
                for k, xt in ((0, xt0), (1, xt1)):
                    for hw_, ps2 in ((0, ps_a), (1, ps_f)):
                        wsl = w1_sb[:, k * 256 + hw_ * 128: k * 256 + hw_ * 128 + 128]
                        for ii in range(2):
                            nc.tensor.matmul(ps2[:, ii, 0:TN], wsl,
                                             xt[:, ii * TN:(ii + 1) * TN],
                                             start=(k == 0), stop=(k == 1))
                act_silu(a_sb[:, i0 * TN:(i0 + 2) * TN].rearrange(
                             "p (g c) -> p g c", g=2),
                         ps_a[:, :, 0:TN], b1_sb[:, 0:1])
                fout = fp3v[:, 1 + 10 * pi: 11 + 10 * pi, 1:1 + W].rearrange(
                    "p (g r) c -> p g r c", g=2)
                act_silu(fout,
                         ps_f[:, :, 0:TN].rearrange("p g (r c) -> p g r c", c=W),
                         b1_sb[:, 1:2], accum_ap=gap_sb[:, pi:pi + 1])

            # ---- router: logits -> softmax -> top-1 one-hot + gate ----
            pooled = spool.tile([CH, 1], f32, tag="pooled")
            nc.vector.reduce_sum(pooled[:], gap_sb[:], axis=AX.X)
            ps_l = psumS.tile([1, NE], f32, tag="psl")
            # wr is pre-scaled by 1/NPIX on the host, so sums (not means) work.
            nc.tensor.matmul(ps_l[:], pooled[:], wr_sb[:], start=True, stop=True)
            logits = spool.tile([1, NE], f32, tag="logits")
            nc.vector.tensor_add(logits[:], ps_l[:], br_sb[:])
            m_sb = spool.tile([1, 1], f32, tag="m")
            nc.vector.reduce_max(m_sb[:], logits[:], axis=AX.X)
            negm = spool.tile([1, 1], f32, tag="negm")
            nc.vector.tensor_scalar_mul(negm[:], m_sb[:], -0.5)
            t_sb = spool.tile([1, NE], f32, tag="tsb")
            nc.scalar.activation(t_sb[:], logits[:], AF.Tanh, bias=negm[:],
                                 scale=0.5)
            num = spool.tile([1, NE], f32, tag="num")
            nc.vector.tensor_scalar_add(num[:], t_sb[:], 1.0)
            den = spool.tile([1, NE], f32, tag="den")
            nc.vector.tensor_scalar(den[:], t_sb[:], -1.0, 1.0,
                                    op0=OP.mult, op1=OP.add)
            rden = spool.tile([1, NE], f32, tag="rden")
            nc.vector.reciprocal(rden[:], den[:])
            e_sb = spool.tile([1, NE], f32, tag="esb")
            nc.vector.tensor_mul(e_sb[:], num[:], rden[:])
            s_sb = spool.tile([1, 1], f32, tag="ssb")
            nc.vector.reduce_sum(s_sb[:], e_sb[:], axis=AX.X)
            wgt = spool.tile([1, 1], f32, tag="wgt")
            nc.vector.reciprocal(wgt[:], s_sb[:])
            oh = spool.tile([1, NE], f32, tag="oh")
            nc.vector.tensor_scalar(oh[:], logits[:], m_sb[:], None, op0=OP.is_ge)
            bc = spool.tile([1, NE + 1], f32, tag="bc")
            nc.vector.tensor_copy(bc[:, 0:NE], oh[:])
            nc.vector.tensor_copy(bc[:, NE:NE + 1], wgt[:])
            ps_bc = psumS.tile([CH, NE + 1], f32, tag="psb")
            nc.tensor.matmul(ps_bc[:], ones_sb[:], bc[:], start=True, stop=True)
            sc = spool.tile([CH, NE + 1], f32, tag="sc")
            nc.vector.tensor_copy(sc[:], ps_bc[:])

            # ---- expert-weight select: Wsel = sum_e onehot[e] * We[e] ----
            wA = selpool.tile([CH, 9 * CH], f32, tag="wA")
            nc.vector.tensor_scalar_mul(wA[:], we_sb[:, 0:1152], sc[:, 0:1])
            wB = selpool.tile([CH, 9 * CH], f32, tag="wB")
            nc.vector.scalar_tensor_tensor(wB[:], we_sb[:, 1152:2304], sc[:, 1:2],
                                           wA[:], op0=OP.mult, op1=OP.add)
            wS = selpool.tile([CH, 9 * CH], dmm, tag="wS")
            nc.vector.scalar_tensor_tensor(wS[:], we_sb[:, 2304:3456], sc[:, 2:3],
                                           wB[:], op0=OP.mult, op1=OP.add)
            bA = spool.tile([CH, 1], f32, tag="bA")
            nc.vector.tensor_scalar_mul(bA[:], be_sb[:, 0:1], sc[:, 0:1])
            bB = spool.tile([CH, 1], f32, tag="bB")
            nc.vector.scalar_tensor_tensor(bB[:], be_sb[:, 1:2], sc[:, 1:2],
                                           bA[:], op0=OP.mult, op1=OP.add)
            bS = spool.tile([CH, 1], f32, tag="bS")
            nc.vector.scalar_tensor_tensor(bS[:], be_sb[:, 2:3], sc[:, 2:3],
                                           bB[:], op0=OP.mult, op1=OP.add)

            # prefetch next sample's first x pairs ahead of this sample's
            # y stores in the DMA queue (slots are free: cv1(b) is done)
            if b + 1 < BPC:
                for pp in range(2):
                    xt_cache[(b + 1, pp)] = emit_xt(b + 1, pp)
            else:
                # refill the loop-carried preload tiles for the next rep's
                # first sample (reads of this rep's b=0 are long done)
                for pp in range(2):
                    t0, t1 = xt_pre[pp]
                    nc.sync.dma_start(t0[:], x_d[0, 0, :, pp * 800:(pp + 1) * 800])
                    nc.sync.dma_start(t1[:], x_d[0, 1, :, pp * 800:(pp + 1) * 800])

            def conv_pair(ps2, wsb, pi):
                i0 = 2 * pi
                for t, (dy, dx) in enumerate(TAPS):
                    wt = wsb[:, t * CH:(t + 1) * CH]
                    for ii in range(2):
                        rhs = fp3[:, (i0 + ii) * RPT + dy: (i0 + ii) * RPT + dy + RPT,
                                  dx: dx + W]
                        nc.tensor.matmul(ps2[:, ii, 0:TN], wt, rhs,
                                         start=(t == 0), stop=(t == 8))

            # ---- shared expert 3x3 conv + SiLU (pairs) ----
            for pi in range(NP):
                ps2 = psum.tile([CH, 2, 512], f32, tag="ps")
                conv_pair(ps2, ws_sb, pi)
                act_silu(sh_sb[:, pi * 800:(pi + 1) * 800].rearrange(
                             "p (g c) -> p g c", g=2),
                         ps2[:, :, 0:TN], bs_sb[:])

            # ---- routed conv + moe + fused cv2, software-pipelined by 1 pair ----
            def cv2_pair(pi):
                i0 = 2 * pi
                for h in range(2):
                    po = psum.tile([CH, 2, 512], f32, tag="ps")
                    for c, src_of in ((0, None), (1, None), (2, None)):
                        wsl = w2_sb[:, c * 256 + h * 128: c * 256 + h * 128 + 128]
                        for ii in range(2):
                            i = i0 + ii
                            if c == 0:
                                rhs = a_sb[:, i * TN:(i + 1) * TN]
                            elif c == 1:
                                rhs = fp3[:, i * RPT + 1: i * RPT + 1 + RPT,
                                          1: 1 + W]
                            else:
                                rhs = moe_sb[:, i * TN:(i + 1) * TN]
                            nc.tensor.matmul(po[:, ii, 0:TN], wsl, rhs,
                                             start=(c == 0), stop=(c == 2))
                    ot = opool.tile([CH, 2 * TN], f32, tag="ot")
                    act_silu(ot[:].rearrange("p (g c) -> p g c", g=2),
                             po[:, :, 0:TN], b2_sb[:, h:h + 1])
                    nc.sync.dma_start(y_d[b, h, :, pi * 800:(pi + 1) * 800], ot[:])

            for pi in range(NP):
                ps2 = psum.tile([CH, 2, 512], f32, tag="ps")
                conv_pair(ps2, wS, pi)
                rt = rpool.tile([CH, 2 * TN], bf16, tag="rt")
                act_silu(rt[:].rearrange("p (g c) -> p g c", g=2),
                         ps2[:, :, 0:TN], bS[:])
                nc.vector.scalar_tensor_tensor(
                    moe_sb[:, pi * 800:(pi + 1) * 800], rt[:], sc[:, NE:NE + 1],
                    sh_sb[:, pi * 800:(pi + 1) * 800], op0=OP.mult, op1=OP.add)
                if pi > 0:
                    cv2_pair(pi - 1)
            cv2_pair(NP - 1)

    if reps == 1:
        _body()
    else:
        # HW timing mode: repeat the whole workload in a hardware loop
        # (same instruction count / compile cost; R x device work).
        with tc.For_i(0, reps, 1):
            _body()
    if internal_io:
        # tiny external output so the (otherwise internal-IO) program is not
        # dead-code eliminated; depends on the looped work via y.
        ydig_d = nc.dram_tensor("ydig", [CH, 4], f32,
                                kind="ExternalOutput").ap()
        ydig_t = opool.tile([CH, 4], f32, name="ydig_t")
        nc.sync.dma_start(ydig_t[:], y_d[0, 0, :, 0:4])
        nc.sync.dma_start(ydig_d, ydig_t[:])


def _ldw_key(ins):
    w = ins.ins[0]
    return (str(getattr(w, "memref", None)), str(getattr(w, "ap", None)),
            getattr(w, "offset", None), str(getattr(w, "dtype", None)),
            str(ins.perf_mode), str(ins.is_transpose))


def dedupe_ldweights(nc):
    """Drop InstLdweights that reload the identical weights as the previous
    Ldweights in the block (adjacent L-M pattern only); move their
    waits/updates onto the immediately-following matmul. The PE keeps the
    stationary operand across matmuls, so the reload is pure overhead."""
    ndrop = 0
    for blk in nc.main_func.blocks:
        out = []
        last_key = None
        pend = None  # dropped ldweights whose sync must move to next matmul
        for ins in blk.instructions:
            if isinstance(ins, mybir.InstLdweights):
                key = _ldw_key(ins)
                if key == last_key and pend is None:
                    pend = ins
                    ndrop += 1
                    continue
                last_key = key
            elif isinstance(ins, mybir.InstMatmult):
                if pend is not None:
                    si, pi = ins.sync_info, pend.sync_info
                    if pi is not None and si is not None:
                        for w in list(pi.on_wait or []):
                            si.on_wait.append(w)
                        for u in list(pi.on_update or []):
                            si.on_update.append(u)
                    pend = None
                if ins.ldweights is not False:
                    last_key = None  # self-loading matmul changes PE weights
            elif pend is not None:
                # something else between the dropped L and its M: bail out,
                # restore the load to stay safe.
                out.append(pend)
                pend = None
                ndrop -= 1
            out.append(ins)
        assert pend is None
        blk.instructions[:] = out
    return ndrop


def build(reps=1, sim_compat=False, tune=None, internal_io=False):
    from contextlib import ExitStack
    nc = bacc.Bacc("TRN2", target_bir_lowering=False, debug=False,
                   num_devices=NCORES)
    with tile.TileContext(nc) as tc:
        with ExitStack() as ctx:
            _emit(nc, tc, ctx, reps=reps, sim_compat=sim_compat, tune=tune,
                  internal_io=internal_io)
    nc.compile()
    dedupe_ldweights(nc)
    return nc


def round_f32r(a):
    """Round fp32 to the PE's fp32r format: 11 explicit mantissa bits
    (round-to-nearest-even), low 12 bits zero. The result is both a valid
    fp32 value and a valid fp32r bit pattern."""
    a = np.ascontiguousarray(np.asarray(a, np.float32))
    bits = a.view(np.uint32).astype(np.uint64)
    lsb = (bits >> 12) & 1
    r = (bits + 0x7FF + lsb) & 0xFFFFF000
    return r.astype(np.uint32).view(np.float32)


def marshal_inputs(x, w1, b1, wr, br, ws, bs, we, be, w2, b2, use_bf16=True):
    """Host-side (tiny) weight re-layouts into matmul-friendly forms."""
    asf = lambda a: np.ascontiguousarray(np.asarray(a, dtype=np.float32))
    if use_bf16:
        import ml_dtypes
        cvt = lambda a: np.ascontiguousarray(
            np.asarray(a, np.float32).astype(ml_dtypes.bfloat16))
    else:
        cvt = round_f32r
    x = cvt(x)
    w1t = asf(np.asarray(w1, np.float32).reshape(2 * CH, C1).T.reshape(2, CH, 2 * CH))
    b1r = asf(np.asarray(b1, np.float32).reshape(2, CH))
    wrs = asf(np.asarray(wr, np.float32) / NPIX)
    brr = asf(np.asarray(br, np.float32).reshape(1, NE))
    wst = asf(np.asarray(ws, np.float32).transpose(1, 2, 3, 0).reshape(CH, 9 * CH))
    bsr = asf(np.asarray(bs, np.float32).reshape(CH, 1))
    wet = asf(np.asarray(we, np.float32).transpose(0, 2, 3, 4, 1).reshape(NE, CH, 9 * CH))
    ber = asf(np.asarray(be, np.float32).T)
    w2t = asf(np.asarray(w2, np.float32).reshape(C2, 3 * CH).T.reshape(3, CH, C2))
    b2r = asf(np.asarray(b2, np.float32).reshape(2, CH))
    w1t = cvt(w1t)
    wst = cvt(wst)
    wet = round_f32r(wet) if not use_bf16 else wet
    w2t = cvt(w2t)
    shared = dict(w1t=w1t, b1r=b1r, wrs=wrs, brr=brr, wst=wst, bsr=bsr,
                  wet=wet, ber=ber, w2t=w2t, b2r=b2r)
    xc = x.reshape(NCORES, BPC, 2, CH, NPIX)
    in_maps = [dict(shared, x=np.ascontiguousarray(xc[c])) for c in range(NCORES)]
    return in_maps


_CACHE = {}


def _get_nc():
    if "nc" not in _CACHE:
        _CACHE["nc"] = build(reps=1)
    return _CACHE["nc"]


def _get_runner():
    """Build the sharded PJRT callable once (mirrors
    bass2jax.run_bass_via_pjrt's multi-core path) so repeat kernel() calls
    skip the jax retrace/compile."""
    if "runner" in _CACHE:
        return _CACHE["runner"]
    import jax
    from jax.experimental.shard_map import shard_map
    from jax.sharding import Mesh, PartitionSpec
    from concourse import bass2jax

    nc = _get_nc()
    bass2jax.install_neuronx_cc_hook()
    part_name = nc.partition_id_tensor.name if nc.partition_id_tensor else None
    in_names, out_names, out_avals = [], [], []
    for alloc in nc.m.functions[0].allocations:
        if not isinstance(alloc, mybir.MemoryLocationSet):
            continue
        name = alloc.memorylocations[0].name
        if alloc.kind == "ExternalInput":
            if name != part_name:
                in_names.append(name)
        elif alloc.kind == "ExternalOutput":
            out_names.append(name)
            out_avals.append(jax.core.ShapedArray(
                tuple(alloc.tensor_shape), mybir.dt.np(alloc.dtype)))
    assert nc.dbg_addr is None
    n_params = len(in_names)
    all_in = in_names + out_names  # zero buffers donated as outputs
    if part_name is not None:
        all_in = all_in + [part_name]

    def _body(*args):
        operands = list(args)
        if part_name is not None:
            operands.append(bass2jax.partition_id_tensor())
        outs = bass2jax._bass_exec_p.bind(
            *operands, out_avals=tuple(out_avals), in_names=tuple(all_in),
            out_names=tuple(out_names), lowering_input_output_aliases=(),
            sim_require_finite=True, sim_require_nnan=True, nc=nc)
        return tuple(outs)

    devices = jax.devices()[:NCORES]
    mesh = Mesh(np.asarray(devices), ("core",))
    nio = n_params + len(out_names)
    sharded = jax.jit(
        shard_map(_body, mesh=mesh, in_specs=(PartitionSpec("core"),) * nio,
                  out_specs=(PartitionSpec("core"),) * len(out_names),
                  check_rep=False),
        donate_argnums=tuple(range(n_params, nio)), keep_unused=True)
    _CACHE["runner"] = (sharded, in_names, out_names, out_avals)
    return _CACHE["runner"]


def kernel(x, w1, b1, wr, br, ws, bs, we, be, w2, b2):
    in_maps = marshal_inputs(x, w1, b1, wr, br, ws, bs, we, be, w2, b2)
    sharded, in_names, out_names, out_avals = _get_runner()
    concat_in = [
        np.concatenate([in_maps[c][name] for c in range(NCORES)], axis=0)
        for name in in_names
    ]
    concat_zeros = [
        np.zeros((NCORES * a.shape[0], *a.shape[1:]), a.dtype) for a in out_avals
    ]
    out_arrs = sharded(*concat_in, *concat_zeros)
    y = np.asarray(out_arrs[out_names.index("y")])
    return np.ascontiguousarray(y.reshape(B, C2, H, W))

